# revision 8
# baseline (speedup 1.0000x reference)
"""Self-contained Trainium2 Bass kernel for causal attention with relative
position bias (B=4, T=1024, D=1024, H=16, dh=64), SPMD across 8 NeuronCores.

Sharding: core = (batch b = core//2, head-half g = core%2). Each core computes
QKV projections for its 8 heads, causal attention, and a partial output
projection; partials are summed pairwise with an on-device AllReduce.

Layouts (per core):
  xT    [128, 8, 1024]  bf16   x[b].T as [d%128, d//128, t]
  wq/wk [128, 8, 512]   bf16   W[:, g*512:+512] as [d%128, d//128, n]
  wv    [128, 8, 512]   bf16   same
  wo    [128, 4, 1024]  bf16   Wo[g*512:+512, :] as [n%128, n//128, m]
  dbias [8, 128, 1024]  f32    per local head: bias[j%128, i-128jb] with
                               causal mask folded in as -1e9
  QT/KT [128, 4, 1024]  bf16   [n%128, n//128, t]  (channel-major)
  V_aug [128, 8, 8, 65] bf16   [t%128, t//128, h, c] with ones column c=64
  pt    [128, 8, 1024]  bf16   exp((logits+bias)/64), [j%128, jb, i]
  attT  [128, 4, 1024]  bf16   normalized attention output, channel-major
"""
import sys

sys.path.insert(0, "/opt/trn_rl_repo")

import numpy as np
import ml_dtypes

B, T, D = 4, 1024, 1024
H, DH = 16, 64
HL, NL = 8, 512  # local heads / channels per core
NCORES = 8
NEG = -1.0e9

_CACHE = {}


def _logit_tiles():
    """Per-head logit tile list: (jb, i0, width). i-chunks start at 128*jb."""
    tiles = []
    for jb in range(8):
        start = 128 * jb
        i0 = start
        while i0 < T:
            w = min(512, T - i0)
            tiles.append((jb, i0, w))
            i0 += w
    return tiles


LOGIT_TILES = _logit_tiles()  # 12 tiles
assert len(LOGIT_TILES) == 12
# exps needed (local tile idx, inclusive) before AV chunk ci can run
_AV_NEED = {0: 0, 1: 0}
for _idx, (_jb, _i0, _w) in enumerate(LOGIT_TILES):
    if _jb <= 3 and _i0 == 128 * _jb:
        _AV_NEED[0] = max(_AV_NEED[0], _idx)
_AV_NEED[1] = 11


def _build():
    from concourse import bass
    from contextlib import ExitStack

    mybir = bass.mybir
    f32, bf16 = mybir.dt.float32, mybir.dt.bfloat16

    nc = bass.Bass(target_bir_lowering=False, debug=False)
    xT = nc.declare_dram_parameter("xT", [128, 8, T], bf16, isOutput=False)
    wq = nc.declare_dram_parameter("wq", [128, 8, NL], bf16, isOutput=False)
    wk = nc.declare_dram_parameter("wk", [128, 8, NL], bf16, isOutput=False)
    wv = nc.declare_dram_parameter("wv", [128, 8, NL], bf16, isOutput=False)
    wo = nc.declare_dram_parameter("wo", [128, 4, D], bf16, isOutput=False)
    dbias = nc.declare_dram_parameter("dbias", [HL, 128, T], f32, isOutput=False)
    bo_rep = nc.declare_dram_parameter("bo_rep", [128, D], f32, isOutput=False)
    out = nc.declare_dram_parameter("out", [T // 2, D], f32, isOutput=True)

    partial = nc.dram_tensor("partial", [T, D], f32)
    red = nc.dram_tensor("red", [T // 2, D], f32)
    r_dram = nc.dram_tensor("r_dram", [2, 128, 4], f32)
    r2_dram = nc.dram_tensor("r2_dram", [2, 128, 4], f32)

    ctx = ExitStack()
    sem = lambda n: ctx.enter_context(nc.semaphore(n))
    sb = lambda n, shape, dt: ctx.enter_context(nc.sbuf_tensor(n, shape, dt))
    ps = lambda n, shape: ctx.enter_context(nc.psum_tensor(n, shape, f32))

    s_xT = sem("s_xT")
    s_wq = sem("s_wq")
    s_wk = sem("s_wk")
    s_wv = sem("s_wv")
    s_c = sem("s_c")
    s_wo = sem("s_wo")
    s_d = [sem(f"s_d{h}") for h in range(HL)]
    s_pe = sem("s_pe")
    s_dve = sem("s_dve")
    s_act = sem("s_act")
    s_rd = [sem("s_rd0"), sem("s_rd1")]
    s_rp = [sem("s_rp0"), sem("s_rp1")]
    s_r2 = [sem("s_r20"), sem("s_r21")]
    s_rbc = [sem("s_rbc0"), sem("s_rbc1")]
    s_tm = [sem("s_tm0"), sem("s_tm1")]
    s_st = [sem(f"s_st{i}") for i in range(4)]
    s_cc = sem("s_cc")
    s_fin = sem("s_fin")

    xT_sb = sb("xT_sb", [128, 8, T], bf16)
    wq_sb = sb("wq_sb", [128, 8, NL], bf16)
    wk_sb = sb("wk_sb", [128, 8, NL], bf16)
    wv_sb = sb("wv_sb", [128, 8, NL], bf16)
    wo_sb = sb("wo_sb", [128, 4, D], bf16)
    qt_sb = sb("qt_sb", [128, 4, T], bf16)
    kt_sb = sb("kt_sb", [128, 4, T], bf16)
    va_sb = sb("va_sb", [128, 8, HL, 65], bf16)
    pt_sb = sb("pt_sb", [128, 8, T], bf16)
    at_sb = sb("at_sb", [128, 4, T], bf16)
    db_sb = [sb("db_sb0", [128, T], f32), sb("db_sb1", [128, T], f32)]
    bo_sb = sb("bo_sb", [128, D], f32)
    sR = [sb("sR0", [128, 512], f32), sb("sR1", [128, 512], f32)]
    sP = [sb("sP0", [128, 4], f32), sb("sP1", [128, 4], f32)]
    rbc = [sb("rbc0", [64, 512], f32), sb("rbc1", [64, 512], f32)]
    tmp = [sb("tmp0", [64, 512], bf16), sb("tmp1", [64, 512], bf16)]
    stg = [sb(f"stg{i}", [128, 512], f32) for i in range(4)]

    ps_mm = [ps("ps_mm0", [128, 512]), ps("ps_mm1", [128, 512])]
    ps_lg = [ps("ps_lg0", [128, 512]), ps("ps_lg1", [128, 512]), ps("ps_lg2", [128, 512])]
    ps_at = [ps("ps_at0", [65, 512]), ps("ps_at1", [65, 512]), ps("ps_at2", [65, 512])]

    # ---- plan ----
    ops = {k: [] for k in ("sp", "pe", "dve", "act", "gp")}

    def wait(eng, s, v):
        ops[eng].append(("wait", s, v))

    def op(eng, fn, inc=None):
        ops[eng].append(("op", fn, inc))

    cnt = {"pe": 0, "dve": 0, "act": 0, "rd": [0, 0], "rp": [0, 0], "r2": [0, 0],
           "rbc": [0, 0], "tm": [0, 0], "st": [0, 0, 0, 0]}
    rec = {}

    # --- input DMAs (SP) ---
    for src, dst, sm in ((xT, xT_sb, s_xT), (wq, wq_sb, s_wq), (wk, wk_sb, s_wk), (wv, wv_sb, s_wv)):
        op("sp", (lambda s=src, d=dst: lambda e: e.dma_start(out=d[:], in_=s[:]))(), (sm, 16))
    op("sp", lambda e: e.dma_start(out=bo_sb[:], in_=bo_rep[:]), (s_c, 16))

    # --- DVE memsets: pt invalid regions + V ones column ---
    for jb in range(1, 8):
        op("dve", (lambda j=jb: lambda e: e.memset(pt_sb[:, j, 0:128 * j], 0.0))(), (s_dve, 1))
        cnt["dve"] += 1
    op("dve", lambda e: e.memset(va_sb[:, :, :, 64:65], 1.0), (s_dve, 1))
    cnt["dve"] += 1
    n_pre = cnt["dve"]  # 8

    # --- QKV projections: 24 psum groups of 8 matmuls ---
    qkv = []
    for nb in range(4):
        for tc in range(2):
            qkv.append(("q", nb, tc))
    for nb in range(4):
        for tc in range(2):
            qkv.append(("k", nb, tc))
    for tb in range(8):
        qkv.append(("v", tb, None))

    for g, item in enumerate(qkv):
        slot = ps_mm[g % 2]
        if g == 0:
            wait("pe", s_xT, 16)
            wait("pe", s_wq, 16)
        elif g == 8:
            wait("pe", s_wk, 16)
        elif g == 16:
            wait("pe", s_wv, 16)
        if g >= 2:
            wait("pe", s_dve, rec[("copy", g - 2)])
        kind = item[0]
        for db in range(8):
            st, sp_ = db == 0, db == 7
            if kind == "q" or kind == "k":
                _, nb, tc = item
                w = wq_sb if kind == "q" else wk_sb
                fn = (lambda w=w, nb=nb, tc=tc, db=db, slot=slot, st=st, sp_=sp_: lambda e: e.matmul(
                    slot[:, :], w[:, db, nb * 128:(nb + 1) * 128], xT_sb[:, db, tc * 512:(tc + 1) * 512],
                    start=st, stop=sp_))()
            else:
                _, tb, _n = item
                fn = (lambda tb=tb, db=db, slot=slot, st=st, sp_=sp_: lambda e: e.matmul(
                    slot[:, :], xT_sb[:, db, tb * 128:(tb + 1) * 128], wv_sb[:, db, 0:NL],
                    start=st, stop=sp_))()
            op("pe", fn, (s_pe, 1) if sp_ else None)
        cnt["pe"] += 1
        rec[("mm", g)] = cnt["pe"]

        wait("dve", s_pe, rec[("mm", g)])
        if kind == "q":
            _, nb, tc = item
            fn = (lambda nb=nb, tc=tc, slot=slot: lambda e: e.tensor_copy(
                qt_sb[:, nb, tc * 512:(tc + 1) * 512], slot[:, :]))()
        elif kind == "k":
            _, nb, tc = item
            fn = (lambda nb=nb, tc=tc, slot=slot: lambda e: e.tensor_copy(
                kt_sb[:, nb, tc * 512:(tc + 1) * 512], slot[:, :]))()
        else:
            _, tb, _n = item
            fn = (lambda tb=tb, slot=slot: lambda e: e.tensor_copy(
                va_sb[:, tb, :, 0:64], slot[:, :]))()
        op("dve", fn, (s_dve, 1))
        cnt["dve"] += 1
        rec[("copy", g)] = cnt["dve"]

    # --- attention ---
    L = 0  # global logit tile idx
    C = 0  # global AV chunk idx
    O = 0  # global odd-head chunk idx
    for h in range(HL):
        g2 = h % 2
        nbh = h // 2
        # SP: dbias DMA into slot h%2
        if h >= 2:
            wait("sp", s_dve, rec[("addlast", h - 2)])
        op("sp", (lambda h=h: lambda e: e.dma_start(out=db_sb[h % 2][:], in_=dbias[h, :, :]))(), (s_d[h], 16))

        # logit tiles
        for tloc, (jb, i0, w) in enumerate(LOGIT_TILES):
            slot = ps_lg[L % 3]
            if L >= 3:
                wait("pe", s_act, L - 2)  # exp of tile L-3 done
            if L == 0:
                wait("pe", s_dve, n_pre + 16)  # all QT/KT copies done
            u0 = i0 - 128 * jb
            fn = (lambda g2=g2, nbh=nbh, jb=jb, i0=i0, w=w, slot=slot: lambda e: e.matmul(
                slot[:, 0:w],
                kt_sb[64 * g2:64 * g2 + 64, nbh, 128 * jb:128 * jb + 128],
                qt_sb[64 * g2:64 * g2 + 64, nbh, i0:i0 + w],
                start=True, stop=True))()
            op("pe", fn, (s_pe, 1))
            cnt["pe"] += 1
            rec[("lg", L)] = cnt["pe"]

            wait("dve", s_pe, rec[("lg", L)])
            if tloc == 0:
                wait("dve", s_d[h], 16)
            fn = (lambda h=h, u0=u0, w=w, slot=slot: lambda e: e.tensor_add(
                slot[:, 0:w], slot[:, 0:w], db_sb[h % 2][:, u0:u0 + w]))()
            op("dve", fn, (s_dve, 1))
            cnt["dve"] += 1
            rec[("add", L)] = cnt["dve"]
            if tloc == len(LOGIT_TILES) - 1:
                rec[("addlast", h)] = cnt["dve"]

            if tloc == 0 and h >= 1:
                wait("act", s_pe, rec[("avdone", h - 1)])
            wait("act", s_dve, rec[("add", L)])
            fn = (lambda jb=jb, i0=i0, w=w, slot=slot: lambda e: e.activation(
                pt_sb[:, jb, i0:i0 + w], slot[:, 0:w],
                bass.mybir.ActivationFunctionType.Exp, scale=1.0 / 64.0))()
            op("act", fn, (s_act, 1))
            cnt["act"] += 1
            L += 1

        # AV chunks
        for ci in range(2):
            slot = ps_at[C % 3]
            wait("pe", s_act, 12 * h + _AV_NEED[ci] + 1)
            if C == 0:
                wait("pe", s_dve, n_pre + 24)  # V_aug ready
            if C >= 3:
                wait("pe", s_dve, rec[("norm", C - 3)])
            jbs = range(4) if ci == 0 else range(8)
            njb = len(list(jbs))
            for k, jb in enumerate(jbs):
                st, sp_ = k == 0, k == njb - 1
                fn = (lambda h=h, jb=jb, ci=ci, slot=slot, st=st, sp_=sp_: lambda e: e.matmul(
                    slot[:, :], va_sb[:, jb, h, 0:65], pt_sb[:, jb, ci * 512:(ci + 1) * 512],
                    start=st, stop=sp_))()
                op("pe", fn, (s_pe, 1) if sp_ else None)
            cnt["pe"] += 1
            rec[("av", C)] = cnt["pe"]
            if ci == 1:
                rec[("avdone", h)] = cnt["pe"]

            # DVE: copy s row to SBUF (slot C%2)
            cs = C % 2
            wait("dve", s_pe, rec[("av", C)])
            if C >= 2:
                wait("dve", s_rd[cs], 16 * (C // 2))  # sR slot free (gp r-dma done)
            op("dve", (lambda cs=cs, slot=slot: lambda e: e.tensor_copy(sR[cs][64:65, :], slot[64:65, :]))(), (s_dve, 1))
            cnt["dve"] += 1
            rec[("scp", C)] = cnt["dve"]

            # GP: sR -> r_dram -> sP[128,4]
            wait("gp", s_dve, rec[("scp", C)])
            if C >= 2:
                wait("gp", s_rp[cs], 16 * (C // 2))  # r_dram slot free
            op("gp", (lambda cs=cs: lambda e: e.dma_start(out=r_dram[cs, :, :], in_=sR[cs][64:65, :]))(), (s_rd[cs], 16))
            cnt["rd"][cs] += 1
            wait("gp", s_rd[cs], 16 * cnt["rd"][cs])
            if C >= 2:
                wait("gp", s_r2[cs], 16 * (C // 2))  # sP slot free
            op("gp", (lambda cs=cs: lambda e: e.dma_start(out=sP[cs][:, :], in_=r_dram[cs, :, :]))(), (s_rp[cs], 16))
            cnt["rp"][cs] += 1

            def plan_recip(Cr):
                csr = Cr % 2
                wait("dve", s_rp[csr], 16 * (Cr // 2 + 1))
                op("dve", (lambda csr=csr: lambda e: e.reciprocal(sP[csr][:, :], sP[csr][:, :]))(), (s_dve, 1))
                cnt["dve"] += 1
                rec[("rcp", Cr)] = cnt["dve"]
                # GP: sP -> r2_dram -> rbc broadcast
                wait("gp", s_dve, rec[("rcp", Cr)])
                if Cr >= 2:
                    wait("gp", s_rbc[csr], 16 * (Cr // 2))  # r2_dram slot free
                op("gp", (lambda csr=csr: lambda e: e.dma_start(out=r2_dram[csr, :, :], in_=sP[csr][:, :]))(), (s_r2[csr], 16))
                cnt["r2"][csr] += 1
                wait("gp", s_r2[csr], 16 * cnt["r2"][csr])
                if ("norm", Cr - 2) in rec:
                    wait("gp", s_dve, rec[("norm", Cr - 2)])  # rbc slot free
                op("gp", (lambda csr=csr: lambda e: e.dma_start(
                    out=rbc[csr][:, :], in_=r2_dram[csr, None, :, :].broadcast_to([64, 128, 4])))(), (s_rbc[csr], 16))
                cnt["rbc"][csr] += 1
                rec[("bcast", Cr)] = (csr, 16 * cnt["rbc"][csr])

            # DVE: norm for chunk Cn (software-pipelined two behind)
            def plan_norm(Cn):
                hn, cin = Cn // 2, Cn % 2
                csn = Cn % 2
                scs, sval = rec[("bcast", Cn)]
                wait("dve", s_rbc[scs], sval)
                slotn = ps_at[Cn % 3]
                if hn % 2 == 0:
                    fn = (lambda hn=hn, cin=cin, csn=csn, slotn=slotn: lambda e: e.tensor_mul(
                        at_sb[0:64, hn // 2, cin * 512:(cin + 1) * 512], slotn[0:64, :], rbc[csn][:, :]))()
                    op("dve", fn, (s_dve, 1))
                else:
                    On = rec[("oidx", Cn)]
                    ts = On % 2
                    if On >= 2:
                        wait("dve", s_tm[ts], 16 * (On // 2))  # tmp slot free
                    fn = (lambda csn=csn, ts=ts, slotn=slotn: lambda e: e.tensor_mul(
                        tmp[ts][:, :], slotn[0:64, :], rbc[csn][:, :]))()
                    op("dve", fn, (s_dve, 1))
                cnt["dve"] += 1
                rec[("norm", Cn)] = cnt["dve"]
                # GP: odd-head shift DMA tmp -> attT
                if hn % 2 == 1:
                    On = rec[("oidx", Cn)]
                    ts = On % 2
                    wait("gp", s_dve, rec[("norm", Cn)])
                    fn = (lambda hn=hn, cin=cin, ts=ts: lambda e: e.dma_start(
                        out=at_sb[64:128, hn // 2, cin * 512:(cin + 1) * 512], in_=tmp[ts][:, :]))()
                    op("gp", fn, (s_tm[ts], 16))
                    cnt["tm"][ts] += 1
                    rec[("tmdma", Cn)] = (ts, 16 * cnt["tm"][ts])

            if h % 2 == 1:
                rec[("oidx", C)] = O
                O += 1
            if C >= 1:
                plan_recip(C - 1)
            if C >= 2:
                plan_norm(C - 2)
            C += 1

        if h == 5:
            op("sp", lambda e: e.dma_start(out=wo_sb[:], in_=wo[:]), (s_wo, 16))

    plan_recip(15)
    plan_norm(14)
    plan_norm(15)

    # --- output projection ---
    for j in range(16):
        tb, mc = j // 2, j % 2
        slot = ps_mm[j % 2]
        if j == 0:
            wait("pe", s_wo, 16)
            wait("pe", s_dve, rec[("norm", 15)])
            for Cn in range(16):
                if (Cn // 2) % 2 == 1:
                    ts, tv = rec[("tmdma", Cn)]
                    wait("pe", s_tm[ts], tv)
        if j >= 2:
            wait("pe", s_dve, rec[("stage", j - 2)])
        for nb in range(4):
            st, sp_ = nb == 0, nb == 3
            fn = (lambda nb=nb, tb=tb, mc=mc, slot=slot, st=st, sp_=sp_: lambda e: e.matmul(
                slot[:, :], at_sb[:, nb, tb * 128:(tb + 1) * 128], wo_sb[:, nb, mc * 512:(mc + 1) * 512],
                start=st, stop=sp_))()
            op("pe", fn, (s_pe, 1) if sp_ else None)
        cnt["pe"] += 1
        rec[("op", j)] = cnt["pe"]

        ss = j % 4
        wait("dve", s_pe, rec[("op", j)])
        if j == 0:
            wait("dve", s_c, 16)
        if j >= 4:
            wait("dve", s_st[ss], 16 * (j // 4))
        fn = (lambda ss=ss, mc=mc, slot=slot: lambda e: e.tensor_add(
            stg[ss][:, :], slot[:, :], bo_sb[:, mc * 512:(mc + 1) * 512]))()
        op("dve", fn, (s_dve, 1))
        cnt["dve"] += 1
        rec[("stage", j)] = cnt["dve"]

        wait("sp", s_dve, rec[("stage", j)])
        fn = (lambda ss=ss, tb=tb, mc=mc: lambda e: e.dma_start(
            out=partial[tb * 128:(tb + 1) * 128, mc * 512:(mc + 1) * 512], in_=stg[ss][:, :]))()
        op("sp", fn, (s_st[ss], 16))
        cnt["st"][ss] += 1

    # --- collective + output ---
    for ss in range(4):
        wait("gp", s_st[ss], 16 * cnt["st"][ss])
    op("gp", lambda e: e.collective_compute(
        "ReduceScatter", bass.mybir.AluOpType.add,
        replica_groups=[[0, 1], [2, 3], [4, 5], [6, 7]],
        ins=[partial.ap().opt()], outs=[red.ap().opt()]), (s_cc, 1))
    wait("gp", s_cc, 1)
    op("gp", lambda e: e.dma_start(out=out[:, :], in_=red[:, :]), (s_fin, 16))
    wait("gp", s_fin, 16)

    # ---- emit ----
    def emit(eng, lst):
        for item in lst:
            if item[0] == "wait":
                eng.wait_ge(item[1], item[2])
            else:
                inst = item[1](eng)
                if item[2] is not None:
                    inst.then_inc(item[2][0], item[2][1])

    with nc.Block() as block:
        @block.sync
        def _(e):
            emit(e, ops["sp"])

        @block.tensor
        def _(e):
            emit(e, ops["pe"])

        @block.vector
        def _(e):
            emit(e, ops["dve"])

        @block.scalar
        def _(e):
            emit(e, ops["act"])

        @block.gpsimd
        def _(e):
            emit(e, ops["gp"])

    ctx.close()
    return nc


def _get_nc():
    if "nc" not in _CACHE:
        _CACHE["nc"] = _build()
    return _CACHE["nc"]


def _prep_inputs(x, Wq, Wk, Wv, Wo, bo, rel_pos_bias):
    bf = ml_dtypes.bfloat16
    in_maps = []
    p_idx = np.arange(128)[:, None]
    u_idx = np.arange(T)[None, :]
    for core in range(NCORES):
        b, g = core // 2, core % 2
        xb = np.asarray(x[b], dtype=np.float32)
        xT_h = np.ascontiguousarray(
            xb.T.reshape(8, 128, T).transpose(1, 0, 2)).astype(bf)
        wq_h = np.ascontiguousarray(
            Wq[:, g * NL:(g + 1) * NL].reshape(8, 128, NL).transpose(1, 0, 2)).astype(bf)
        wk_h = np.ascontiguousarray(
            Wk[:, g * NL:(g + 1) * NL].reshape(8, 128, NL).transpose(1, 0, 2)).astype(bf)
        wv_h = np.ascontiguousarray(
            Wv[:, g * NL:(g + 1) * NL].reshape(8, 128, NL).transpose(1, 0, 2)).astype(bf)
        wo_h = np.ascontiguousarray(
            Wo[g * NL:(g + 1) * NL, :].reshape(4, 128, D).transpose(1, 0, 2)).astype(bf)
        db = np.empty((HL, 128, T), dtype=np.float32)
        for h in range(HL):
            rev = np.asarray(rel_pos_bias[g * HL + h], dtype=np.float32)[::-1]
            dif = np.clip(u_idx - p_idx, 0, T - 1)
            db[h] = np.where(u_idx >= p_idx, rev[dif], NEG)
        bo_h = np.broadcast_to(np.asarray(bo, np.float32) * 0.5, (128, D)).copy()
        in_maps.append({
            "xT": xT_h, "wq": wq_h, "wk": wk_h, "wv": wv_h, "wo": wo_h,
            "dbias": db, "bo_rep": bo_h,
        })
    return in_maps


def run_on_device(x, Wq, Wk, Wv, Wo, bo, rel_pos_bias, trace=False):
    from concourse.bass_utils import run_bass_kernel_spmd

    nc = _get_nc()
    in_maps = _prep_inputs(x, Wq, Wk, Wv, Wo, bo, rel_pos_bias)
    res = run_bass_kernel_spmd(nc, in_maps, core_ids=list(range(NCORES)), trace=trace)
    out = np.stack([
        np.concatenate([res.results[2 * b]["out"], res.results[2 * b + 1]["out"]], axis=0)
        for b in range(B)
    ]).astype(np.float32)
    return out, res


def kernel(x, Wq, Wk, Wv, Wo, bo, rel_pos_bias):
    out, _ = run_on_device(x, Wq, Wk, Wv, Wo, bo, rel_pos_bias, trace=False)
    return out


# revision 12
# speedup vs baseline: 1.1956x; 1.1956x over previous
"""Self-contained Trainium2 Bass kernel for causal attention with relative
position bias (B=4, T=1024, D=1024, H=16, dh=64), SPMD across 8 NeuronCores.

Sharding: core = (batch b = core//2, head-half g = core%2). Each core computes
QKV projections for its 8 heads, causal attention, and a partial output
projection; partials are summed pairwise with an on-device AllReduce.

Layouts (per core):
  xT    [128, 8, 1024]  bf16   x[b].T as [d%128, d//128, t]
  wq/wk [128, 8, 512]   bf16   W[:, g*512:+512] as [d%128, d//128, n]
  wv    [128, 8, 512]   bf16   same
  wo    [128, 4, 1024]  bf16   Wo[g*512:+512, :] as [n%128, n//128, m]
  dbias [8, 128, 1024]  f32    per local head: bias[j%128, i-128jb] with
                               causal mask folded in as -1e9
  QT/KT [128, 4, 1024]  bf16   [n%128, n//128, t]  (channel-major)
  V_aug [128, 8, 8, 65] bf16   [t%128, t//128, h, c] with ones column c=64
  pt    [128, 8, 1024]  bf16   exp((logits+bias)/64), [j%128, jb, i]
  attT  [128, 4, 1024]  bf16   normalized attention output, channel-major
"""
import sys

sys.path.insert(0, "/opt/trn_rl_repo")

import numpy as np
import ml_dtypes

B, T, D = 4, 1024, 1024
H, DH = 16, 64
HL, NL = 8, 512  # local heads / channels per core
NCORES = 8
NEG = -1.0e9

_CACHE = {}


def _logit_tiles():
    """Per-head logit tile list: (jb, i0, width). i-chunks start at 128*jb."""
    tiles = []
    for jb in range(8):
        start = 128 * jb
        i0 = start
        while i0 < T:
            w = min(512, T - i0)
            tiles.append((jb, i0, w))
            i0 += w
    return tiles


LOGIT_TILES = _logit_tiles()  # 12 tiles
assert len(LOGIT_TILES) == 12
# exps needed (local tile idx, inclusive) before AV chunk ci can run
_AV_NEED = {0: 0, 1: 0}
for _idx, (_jb, _i0, _w) in enumerate(LOGIT_TILES):
    if _jb <= 3 and _i0 == 128 * _jb:
        _AV_NEED[0] = max(_AV_NEED[0], _idx)
_AV_NEED[1] = 11


def _build():
    from concourse import bass
    from contextlib import ExitStack

    mybir = bass.mybir
    f32, bf16 = mybir.dt.float32, mybir.dt.bfloat16

    nc = bass.Bass(target_bir_lowering=False, debug=False)
    xT = nc.declare_dram_parameter("xT", [128, 8, T], bf16, isOutput=False)
    wq = nc.declare_dram_parameter("wq", [128, 8, NL], bf16, isOutput=False)
    wk = nc.declare_dram_parameter("wk", [128, 8, NL], bf16, isOutput=False)
    wv = nc.declare_dram_parameter("wv", [128, 8, NL], bf16, isOutput=False)
    wo = nc.declare_dram_parameter("wo", [128, 4, D], bf16, isOutput=False)
    dbias = nc.declare_dram_parameter("dbias", [HL, 128, T], f32, isOutput=False)
    bo_rep = nc.declare_dram_parameter("bo_rep", [128, D], f32, isOutput=False)
    out = nc.declare_dram_parameter("out", [T // 2, D], bf16, isOutput=True)

    partial = nc.dram_tensor("partial", [T, D], bf16)
    red = nc.dram_tensor("red", [T // 2, D], bf16)
    r_dram = nc.dram_tensor("r_dram", [2, 128, 4], f32)
    r2_dram = nc.dram_tensor("r2_dram", [2, 128, 4], f32)

    ctx = ExitStack()
    sem = lambda n: ctx.enter_context(nc.semaphore(n))
    sb = lambda n, shape, dt: ctx.enter_context(nc.sbuf_tensor(n, shape, dt))
    ps = lambda n, shape: ctx.enter_context(nc.psum_tensor(n, shape, f32))

    s_xT = sem("s_xT")
    s_wq = sem("s_wq")
    s_wk = sem("s_wk")
    s_wv = sem("s_wv")
    s_c = sem("s_c")
    s_wo = sem("s_wo")
    s_d = [sem(f"s_d{h}") for h in range(HL)]
    s_pe = sem("s_pe")
    s_dve = sem("s_dve")
    s_act = sem("s_act")
    s_rd = [sem("s_rd0"), sem("s_rd1")]
    s_rp = [sem("s_rp0"), sem("s_rp1")]
    s_r2 = [sem("s_r20"), sem("s_r21")]
    s_rbc = [sem("s_rbc0"), sem("s_rbc1")]
    s_tm = [sem("s_tm0"), sem("s_tm1")]
    s_st = [sem(f"s_st{i}") for i in range(4)]
    s_out = sem("s_out")
    s_cc = sem("s_cc")
    s_fin = sem("s_fin")

    xT_sb = sb("xT_sb", [128, 8, T], bf16)
    wq_sb = sb("wq_sb", [128, 8, NL], bf16)
    wk_sb = sb("wk_sb", [128, 8, NL], bf16)
    wv_sb = sb("wv_sb", [128, 8, NL], bf16)
    wo_sb = sb("wo_sb", [128, 4, D], bf16)
    qt_sb = sb("qt_sb", [128, 4, T], bf16)
    kt_sb = sb("kt_sb", [128, 4, T], bf16)
    va_sb = sb("va_sb", [128, 8, HL, 65], bf16)
    pt_sb = sb("pt_sb", [128, 8, T], bf16)
    at_sb = sb("at_sb", [128, 4, T], bf16)
    db_sb = [sb("db_sb0", [128, T], f32), sb("db_sb1", [128, T], f32)]
    bo_sb = sb("bo_sb", [128, D], f32)
    sR = [sb("sR0", [128, 512], f32), sb("sR1", [128, 512], f32)]
    sP = [sb("sP0", [128, 4], f32), sb("sP1", [128, 4], f32)]
    rbc = [sb("rbc0", [64, 512], f32), sb("rbc1", [64, 512], f32)]
    tmp = [sb("tmp0", [64, 512], bf16), sb("tmp1", [64, 512], bf16)]
    stg_own = sb("stg_own", [128, 16, 512], bf16)

    ps_mm = [ps("ps_mm0", [128, 512]), ps("ps_mm1", [128, 512])]
    ps_lg = [ps("ps_lg0", [128, 512]), ps("ps_lg1", [128, 512]), ps("ps_lg2", [128, 512])]
    ps_at = [ps("ps_at0", [65, 512]), ps("ps_at1", [65, 512]), ps("ps_at2", [65, 512])]

    # ---- plan ----
    ops = {k: [] for k in ("sp", "pe", "dve", "act", "gp")}

    def wait(eng, s, v):
        ops[eng].append(("wait", s, v))

    def op(eng, fn, inc=None):
        ops[eng].append(("op", fn, inc))

    cnt = {"pe": 0, "dve": 0, "act": 0, "rd": [0, 0], "rp": [0, 0], "r2": [0, 0],
           "rbc": [0, 0], "tm": [0, 0], "st": [0, 0, 0, 0]}
    rec = {}

    # --- input DMAs (SP) ---
    for src, dst, sm in ((xT, xT_sb, s_xT), (wq, wq_sb, s_wq), (wk, wk_sb, s_wk), (wv, wv_sb, s_wv)):
        op("sp", (lambda s=src, d=dst: lambda e: e.dma_start(out=d[:], in_=s[:]))(), (sm, 16))
    op("sp", lambda e: e.dma_start(out=bo_sb[:], in_=bo_rep[:]), (s_c, 16))

    # --- DVE memsets: pt invalid regions + V ones column ---
    for jb in range(1, 8):
        op("dve", (lambda j=jb: lambda e: e.memset(pt_sb[:, j, 0:128 * j], 0.0))(), (s_dve, 1))
        cnt["dve"] += 1
    op("dve", lambda e: e.memset(va_sb[:, :, :, 64:65], 1.0), (s_dve, 1))
    cnt["dve"] += 1
    n_pre = cnt["dve"]  # 8

    # --- QKV projections: 24 psum groups of 8 matmuls ---
    qkv = []
    for nb in range(4):
        for tc in range(2):
            qkv.append(("q", nb, tc))
    for nb in range(4):
        for tc in range(2):
            qkv.append(("k", nb, tc))
    for tb in range(8):
        qkv.append(("v", tb, None))

    for g, item in enumerate(qkv):
        slot = ps_mm[g % 2]
        if g == 0:
            wait("pe", s_xT, 16)
            wait("pe", s_wq, 16)
        elif g == 8:
            wait("pe", s_wk, 16)
        elif g == 16:
            wait("pe", s_wv, 16)
        if g >= 2:
            wait("pe", s_dve, rec[("copy", g - 2)])
        kind = item[0]
        for db in range(8):
            st, sp_ = db == 0, db == 7
            if kind == "q" or kind == "k":
                _, nb, tc = item
                w = wq_sb if kind == "q" else wk_sb
                fn = (lambda w=w, nb=nb, tc=tc, db=db, slot=slot, st=st, sp_=sp_: lambda e: e.matmul(
                    slot[:, :], w[:, db, nb * 128:(nb + 1) * 128], xT_sb[:, db, tc * 512:(tc + 1) * 512],
                    start=st, stop=sp_))()
            else:
                _, tb, _n = item
                fn = (lambda tb=tb, db=db, slot=slot, st=st, sp_=sp_: lambda e: e.matmul(
                    slot[:, :], xT_sb[:, db, tb * 128:(tb + 1) * 128], wv_sb[:, db, 0:NL],
                    start=st, stop=sp_))()
            op("pe", fn, (s_pe, 1) if sp_ else None)
        cnt["pe"] += 1
        rec[("mm", g)] = cnt["pe"]

        wait("dve", s_pe, rec[("mm", g)])
        if kind == "q":
            _, nb, tc = item
            fn = (lambda nb=nb, tc=tc, slot=slot: lambda e: e.tensor_copy(
                qt_sb[:, nb, tc * 512:(tc + 1) * 512], slot[:, :]))()
        elif kind == "k":
            _, nb, tc = item
            fn = (lambda nb=nb, tc=tc, slot=slot: lambda e: e.tensor_copy(
                kt_sb[:, nb, tc * 512:(tc + 1) * 512], slot[:, :]))()
        else:
            _, tb, _n = item
            fn = (lambda tb=tb, slot=slot: lambda e: e.tensor_copy(
                va_sb[:, tb, :, 0:64], slot[:, :]))()
        op("dve", fn, (s_dve, 1))
        cnt["dve"] += 1
        rec[("copy", g)] = cnt["dve"]

    # --- attention ---
    L = 0  # global logit tile idx
    C = 0  # global AV chunk idx
    O = 0  # global odd-head chunk idx
    for h in range(HL):
        g2 = h % 2
        nbh = h // 2
        # SP: dbias DMA into slot h%2
        if h >= 2:
            wait("sp", s_dve, rec[("addlast", h - 2)])
        op("sp", (lambda h=h: lambda e: e.dma_start(out=db_sb[h % 2][:], in_=dbias[h, :, :]))(), (s_d[h], 16))

        # logit tiles (5 psum slots: 3 dedicated + 2 borrowed from ps_mm)
        lgslots = ps_lg + ps_mm
        for tloc, (jb, i0, w) in enumerate(LOGIT_TILES):
            slot = lgslots[L % 5]
            if L >= 5:
                wait("pe", s_act, L - 4)  # exp of tile L-5 done
            if L == 0:
                wait("pe", s_dve, n_pre + 16)  # all QT/KT copies done
            elif L == 3:
                wait("pe", s_dve, rec[("copy", 22)])  # ps_mm0 free of QKV
            elif L == 4:
                wait("pe", s_dve, rec[("copy", 23)])  # ps_mm1 free of QKV
            u0 = i0 - 128 * jb
            fn = (lambda g2=g2, nbh=nbh, jb=jb, i0=i0, w=w, slot=slot: lambda e: e.matmul(
                slot[:, 0:w],
                kt_sb[64 * g2:64 * g2 + 64, nbh, 128 * jb:128 * jb + 128],
                qt_sb[64 * g2:64 * g2 + 64, nbh, i0:i0 + w],
                start=True, stop=True))()
            op("pe", fn, (s_pe, 1))
            cnt["pe"] += 1
            rec[("lg", L)] = cnt["pe"]

            wait("dve", s_pe, rec[("lg", L)])
            if tloc == 0:
                wait("dve", s_d[h], 16)
            fn = (lambda h=h, u0=u0, w=w, slot=slot: lambda e: e.tensor_add(
                slot[:, 0:w], slot[:, 0:w], db_sb[h % 2][:, u0:u0 + w]))()
            op("dve", fn, (s_dve, 1))
            cnt["dve"] += 1
            rec[("add", L)] = cnt["dve"]
            if tloc == len(LOGIT_TILES) - 1:
                rec[("addlast", h)] = cnt["dve"]

            if tloc == 0 and h >= 1:
                wait("act", s_pe, rec[("avdone", h - 1)])
            wait("act", s_dve, rec[("add", L)])
            fn = (lambda jb=jb, i0=i0, w=w, slot=slot: lambda e: e.activation(
                pt_sb[:, jb, i0:i0 + w], slot[:, 0:w],
                bass.mybir.ActivationFunctionType.Exp, scale=1.0 / 64.0))()
            op("act", fn, (s_act, 1))
            cnt["act"] += 1
            L += 1

        # AV chunks
        for ci in range(2):
            slot = ps_at[C % 3]
            wait("pe", s_act, 12 * h + _AV_NEED[ci] + 1)
            if C == 0:
                wait("pe", s_dve, n_pre + 24)  # V_aug ready
            if C >= 3:
                wait("pe", s_dve, rec[("norm", C - 3)])
            jbs = range(4) if ci == 0 else range(8)
            njb = len(list(jbs))
            for k, jb in enumerate(jbs):
                st, sp_ = k == 0, k == njb - 1
                fn = (lambda h=h, jb=jb, ci=ci, slot=slot, st=st, sp_=sp_: lambda e: e.matmul(
                    slot[:, :], va_sb[:, jb, h, 0:65], pt_sb[:, jb, ci * 512:(ci + 1) * 512],
                    start=st, stop=sp_))()
                op("pe", fn, (s_pe, 1) if sp_ else None)
            cnt["pe"] += 1
            rec[("av", C)] = cnt["pe"]
            if ci == 1:
                rec[("avdone", h)] = cnt["pe"]

            # DVE: copy s row to SBUF (slot C%2)
            cs = C % 2
            wait("dve", s_pe, rec[("av", C)])
            if C >= 2:
                wait("dve", s_rd[cs], 16 * (C // 2))  # sR slot free (gp r-dma done)
            op("dve", (lambda cs=cs, slot=slot: lambda e: e.tensor_copy(sR[cs][64:65, :], slot[64:65, :]))(), (s_dve, 1))
            cnt["dve"] += 1
            rec[("scp", C)] = cnt["dve"]

            # GP: sR -> r_dram -> sP[128,4]
            wait("gp", s_dve, rec[("scp", C)])
            if C >= 2:
                wait("gp", s_rp[cs], 16 * (C // 2))  # r_dram slot free
            op("gp", (lambda cs=cs: lambda e: e.dma_start(out=r_dram[cs, :, :], in_=sR[cs][64:65, :]))(), (s_rd[cs], 16))
            cnt["rd"][cs] += 1
            wait("gp", s_rd[cs], 16 * cnt["rd"][cs])
            if C >= 2:
                wait("gp", s_r2[cs], 16 * (C // 2))  # sP slot free
            op("gp", (lambda cs=cs: lambda e: e.dma_start(out=sP[cs][:, :], in_=r_dram[cs, :, :]))(), (s_rp[cs], 16))
            cnt["rp"][cs] += 1

            def plan_recip(Cr):
                csr = Cr % 2
                wait("dve", s_rp[csr], 16 * (Cr // 2 + 1))
                op("dve", (lambda csr=csr: lambda e: e.reciprocal(sP[csr][:, :], sP[csr][:, :]))(), (s_dve, 1))
                cnt["dve"] += 1
                rec[("rcp", Cr)] = cnt["dve"]
                # GP: sP -> r2_dram -> rbc broadcast
                wait("gp", s_dve, rec[("rcp", Cr)])
                if Cr >= 2:
                    wait("gp", s_rbc[csr], 16 * (Cr // 2))  # r2_dram slot free
                op("gp", (lambda csr=csr: lambda e: e.dma_start(out=r2_dram[csr, :, :], in_=sP[csr][:, :]))(), (s_r2[csr], 16))
                cnt["r2"][csr] += 1
                wait("gp", s_r2[csr], 16 * cnt["r2"][csr])
                if ("norm", Cr - 2) in rec:
                    wait("gp", s_dve, rec[("norm", Cr - 2)])  # rbc slot free
                op("gp", (lambda csr=csr: lambda e: e.dma_start(
                    out=rbc[csr][:, :], in_=r2_dram[csr, None, :, :].broadcast_to([64, 128, 4])))(), (s_rbc[csr], 16))
                cnt["rbc"][csr] += 1
                rec[("bcast", Cr)] = (csr, 16 * cnt["rbc"][csr])

            # DVE: norm for chunk Cn (software-pipelined two behind)
            def plan_norm(Cn):
                hn, cin = Cn // 2, Cn % 2
                csn = Cn % 2
                scs, sval = rec[("bcast", Cn)]
                wait("dve", s_rbc[scs], sval)
                slotn = ps_at[Cn % 3]
                if hn % 2 == 0:
                    fn = (lambda hn=hn, cin=cin, csn=csn, slotn=slotn: lambda e: e.tensor_mul(
                        at_sb[0:64, hn // 2, cin * 512:(cin + 1) * 512], slotn[0:64, :], rbc[csn][:, :]))()
                    op("dve", fn, (s_dve, 1))
                else:
                    On = rec[("oidx", Cn)]
                    ts = On % 2
                    if On >= 2:
                        wait("dve", s_tm[ts], 16 * (On // 2))  # tmp slot free
                    fn = (lambda csn=csn, ts=ts, slotn=slotn: lambda e: e.tensor_mul(
                        tmp[ts][:, :], slotn[0:64, :], rbc[csn][:, :]))()
                    op("dve", fn, (s_dve, 1))
                cnt["dve"] += 1
                rec[("norm", Cn)] = cnt["dve"]
                # GP: odd-head shift DMA tmp -> attT
                if hn % 2 == 1:
                    On = rec[("oidx", Cn)]
                    ts = On % 2
                    wait("gp", s_dve, rec[("norm", Cn)])
                    fn = (lambda hn=hn, cin=cin, ts=ts: lambda e: e.dma_start(
                        out=at_sb[64:128, hn // 2, cin * 512:(cin + 1) * 512], in_=tmp[ts][:, :]))()
                    op("gp", fn, (s_tm[ts], 16))
                    cnt["tm"][ts] += 1
                    rec[("tmdma", Cn)] = (ts, 16 * cnt["tm"][ts])

            if h % 2 == 1:
                rec[("oidx", C)] = O
                O += 1
            if C >= 1:
                plan_recip(C - 1)
            if C >= 2:
                plan_norm(C - 2)
            C += 1

        if h == 5:
            op("sp", lambda e: e.dma_start(out=wo_sb[:], in_=wo[:]), (s_wo, 16))

    plan_recip(15)
    plan_norm(14)
    plan_norm(15)

    # --- output projection + peer reduce (remote DMA to pair-mate) ---
    n_prep = 0
    for j in range(16):
        tb, mc = j // 2, j % 2
        slot = ps_mm[j % 2]
        if j == 0:
            wait("pe", s_wo, 16)
            wait("pe", s_dve, rec[("norm", 15)])
            for Cn in range(16):
                if (Cn // 2) % 2 == 1:
                    ts, tv = rec[("tmdma", Cn)]
                    wait("pe", s_tm[ts], tv)
        if j >= 2:
            wait("pe", s_dve, rec[("stage", j - 2)])
        for nb in range(4):
            st, sp_ = nb == 0, nb == 3
            fn = (lambda nb=nb, tb=tb, mc=mc, slot=slot, st=st, sp_=sp_: lambda e: e.matmul(
                slot[:, :], at_sb[:, nb, tb * 128:(tb + 1) * 128], wo_sb[:, nb, mc * 512:(mc + 1) * 512],
                start=st, stop=sp_))()
            op("pe", fn, (s_pe, 1) if sp_ else None)
        cnt["pe"] += 1
        rec[("op", j)] = cnt["pe"]

        wait("dve", s_pe, rec[("op", j)])
        if j == 0:
            wait("dve", s_c, 16)
        fn = (lambda j=j, mc=mc, slot=slot: lambda e: e.tensor_add(
            stg_own[:, j, :], slot[:, :], bo_sb[:, mc * 512:(mc + 1) * 512]))()
        op("dve", fn, (s_dve, 1))
        cnt["dve"] += 1
        rec[("stage", j)] = cnt["dve"]

        wait("sp", s_dve, rec[("stage", j)])
        fn = (lambda j=j, tb=tb, mc=mc: lambda e: e.dma_start(
            out=partial[tb * 128:(tb + 1) * 128, mc * 512:(mc + 1) * 512], in_=stg_own[:, j, :]))()
        op("sp", fn, (s_out, 16))

    # --- pairwise ReduceScatter (bf16) + output ---
    wait("gp", s_out, 16 * 16)
    op("gp", lambda e: e.collective_compute(
        "ReduceScatter", bass.mybir.AluOpType.add,
        replica_groups=[[0, 1], [2, 3], [4, 5], [6, 7]],
        ins=[partial.ap().opt()], outs=[red.ap().opt()]), (s_cc, 1))
    wait("gp", s_cc, 1)
    op("gp", lambda e: e.dma_start(out=out[:, :], in_=red[:, :]), (s_fin, 16))
    wait("gp", s_fin, 16)

    # ---- emit ----
    def emit(eng, lst):
        for item in lst:
            if item[0] == "wait":
                eng.wait_ge(item[1], item[2])
            else:
                inst = item[1](eng)
                if item[2] is not None:
                    inst.then_inc(item[2][0], item[2][1])

    with nc.Block() as block:
        @block.sync
        def _(e):
            emit(e, ops["sp"])

        @block.tensor
        def _(e):
            emit(e, ops["pe"])

        @block.vector
        def _(e):
            emit(e, ops["dve"])

        @block.scalar
        def _(e):
            emit(e, ops["act"])

        @block.gpsimd
        def _(e):
            emit(e, ops["gp"])

    ctx.close()
    return nc


def _get_nc():
    if "nc" not in _CACHE:
        _CACHE["nc"] = _build()
    return _CACHE["nc"]


def _prep_inputs(x, Wq, Wk, Wv, Wo, bo, rel_pos_bias):
    bf = ml_dtypes.bfloat16
    in_maps = []
    p_idx = np.arange(128)[:, None]
    u_idx = np.arange(T)[None, :]
    for core in range(NCORES):
        b, g = core // 2, core % 2
        xb = np.asarray(x[b], dtype=np.float32)
        xT_h = np.ascontiguousarray(
            xb.T.reshape(8, 128, T).transpose(1, 0, 2)).astype(bf)
        wq_h = np.ascontiguousarray(
            Wq[:, g * NL:(g + 1) * NL].reshape(8, 128, NL).transpose(1, 0, 2)).astype(bf)
        wk_h = np.ascontiguousarray(
            Wk[:, g * NL:(g + 1) * NL].reshape(8, 128, NL).transpose(1, 0, 2)).astype(bf)
        wv_h = np.ascontiguousarray(
            Wv[:, g * NL:(g + 1) * NL].reshape(8, 128, NL).transpose(1, 0, 2)).astype(bf)
        wo_h = np.ascontiguousarray(
            Wo[g * NL:(g + 1) * NL, :].reshape(4, 128, D).transpose(1, 0, 2)).astype(bf)
        db = np.empty((HL, 128, T), dtype=np.float32)
        for h in range(HL):
            rev = np.asarray(rel_pos_bias[g * HL + h], dtype=np.float32)[::-1]
            dif = np.clip(u_idx - p_idx, 0, T - 1)
            db[h] = np.where(u_idx >= p_idx, rev[dif], NEG)
        bo_h = np.broadcast_to(np.asarray(bo, np.float32) * 0.5, (128, D)).copy()
        in_maps.append({
            "xT": xT_h, "wq": wq_h, "wk": wk_h, "wv": wv_h, "wo": wo_h,
            "dbias": db, "bo_rep": bo_h,
        })
    return in_maps


def run_on_device(x, Wq, Wk, Wv, Wo, bo, rel_pos_bias, trace=False):
    from concourse.bass_utils import run_bass_kernel_spmd

    nc = _get_nc()
    in_maps = _prep_inputs(x, Wq, Wk, Wv, Wo, bo, rel_pos_bias)
    res = run_bass_kernel_spmd(nc, in_maps, core_ids=list(range(NCORES)), trace=trace)
    out = np.stack([
        np.concatenate([res.results[2 * b]["out"], res.results[2 * b + 1]["out"]], axis=0)
        for b in range(B)
    ]).astype(np.float32)
    return out, res


def kernel(x, Wq, Wk, Wv, Wo, bo, rel_pos_bias):
    out, _ = run_on_device(x, Wq, Wk, Wv, Wo, bo, rel_pos_bias, trace=False)
    return out
